# revision 7
# baseline (speedup 1.0000x reference)
"""Causal multi-head attention (B=1, S=4096, H=16, Dh=64) on 8 TRN2
NeuronCores, head-parallel (2 heads per core), flash-style (scores never
touch HBM).

Per-core SPMD program (q/k/v [4096, 128] fp32 = 2 heads side by side,
output o [4096, 128] fp32):
  - Scores transposed, S^T[k, q] = K @ Q^T, contraction dh=64, fp16; the
    two heads sit at partitions 0..63 / 64..127 so their score matmuls
    land on different PE row groups and run concurrently.
  - exp() split across TWO engines, balanced at build time:
      * ScalarE ACT: p = exp(s/8) -> fp16          (~1.11us / block)
      * VectorE DVE: Schraudolph bit-trick exp     (~1.22us / block)
        i16 = round(s * (2^10*log2e/8) + (15*2^10 - 44)); bitcast fp16.
        Max rel err ~3%; softmax ratio cancels most of it (measured
        end-to-end ~5e-3 vs 2e-2 budget).
  - Causality at block granularity: upper-triangle k-blocks skipped;
    diagonal blocks multiplied by 0/1 fp16 masks (one wide [128, 2, w]
    tensor_tensor covering both heads, w bounded per sub-diagonal).
  - AV: out^T[dh, q] per head accumulated in PSUM via lhsT = V_aug
    [128, 65] (V plus ones column -> row 64 = softmax denominator).
  - Masks + identities are DMA'd in as precomputed constants (gpsimd
    iota/affine_select is far too slow to sit on the critical path).
  - Setup pipelined in fine groups: blocks 0-7 of K^T/Q^T built via PE
    transpose bootstrap (main loop starts ~3us), blocks 8-31 via
    fp16 DRAM round trip + xbar DMA transpose, spread over two queues.
"""
import numpy as np

import concourse.bass as bass
import concourse.tile as tile
import concourse.mybir as mybir
from concourse import bacc

FP32 = mybir.dt.float32
FP16 = mybir.dt.float16
I16 = mybir.dt.int16

S = 4096
DH = 64
NHEAD = 2          # heads per core
DCORE = NHEAD * DH
NB = S // 128      # 32 k-blocks
QC = 512
NQC = S // QC      # 8 q-chunks
SCALE = 1.0 / 8.0
EXP = mybir.ActivationFunctionType.Exp

# Schraudolph constants (fp16 target): i16 = s * C1 + C2, bitcast fp16.
SCH_C1 = float(1024.0 * 1.4426950408889634 * SCALE)
SCH_C2 = float(15 * 1024 - 44.0)

_CACHED_NC = None
TRACE = False
LAST_RES = None


def _build_consts():
    """Host-side constant tensors DMA'd into SBUF at kernel start."""
    # wide diagonal masks: cm[:, di, c] for both heads (c mod 512 is the
    # in-head q column); keep iff p <= c - 128*di
    p = np.arange(128)[:, None]
    c = np.arange(512)[None, :]
    masks = np.zeros((128, 4, 2, 512), dtype=np.float16)
    for di in range(4):
        m = (p <= c - 128 * di).astype(np.float16)
        masks[:, di, 0, :] = m
        masks[:, di, 1, :] = m
    ident16 = np.eye(128, dtype=np.float16)
    cm = np.concatenate([masks.reshape(128, 4096), ident16], axis=1)
    ci = np.eye(128, 65, dtype=np.float32)   # fp32 identity for epilogue T
    return cm, ci


def build_attn():
    nc = bacc.Bacc(None, target_bir_lowering=False, debug=False)
    q_d = nc.dram_tensor("q", [S, DCORE], FP32, kind="ExternalInput")
    k_d = nc.dram_tensor("k", [S, DCORE], FP32, kind="ExternalInput")
    v_d = nc.dram_tensor("v", [S, DCORE], FP32, kind="ExternalInput")
    cm_d = nc.dram_tensor("cm", [128, 4096 + 128], FP16, kind="ExternalInput")
    ci_d = nc.dram_tensor("ci", [128, 65], FP32, kind="ExternalInput")
    o_d = nc.dram_tensor("o", [S, DCORE], FP32, kind="ExternalOutput")

    # build-time engine load balancer (ns estimates from HW microbench)
    load = {"s": 2000.0, "v": 11000.0}
    COST_S_EXP, COST_V_EXP = 1150.0, 1250.0
    MASK_COST = (260.0, 400.0, 530.0, 660.0)
    COST_COPY_S, COST_COPY_V = 700.0, 700.0

    with tile.TileContext(nc) as tc:
        with (
            tc.tile_pool(name="cst", bufs=1) as cst,
            tc.tile_pool(name="nat", bufs=3) as nat,
            tc.tile_pool(name="pp", bufs=6) as pp,
            tc.tile_pool(name="ep", bufs=4) as ep,
            tc.tile_pool(name="dram", bufs=1, space="DRAM") as dram,
            tc.tile_pool(name="ps_s", bufs=2, space="PSUM") as ps_s,
            tc.tile_pool(name="ps_o0", bufs=1, space="PSUM") as ps_o0,
            tc.tile_pool(name="ps_o1", bufs=1, space="PSUM") as ps_o1,
            tc.tile_pool(name="ps_t", bufs=2, space="PSUM") as ps_tp,
        ):
            # ---------- constants (DMA'd, not computed) ----------
            cmt = cst.tile([128, 4096 + 128], FP16, tag="cmt")
            ci = cst.tile([128, 65], FP32, tag="ci")
            nc.sync.dma_start(cmt[:], cm_d.ap())
            nc.sync.dma_start(ci[:], ci_d.ap())
            mm = cmt[:, 0:4096].rearrange("p (di h c) -> p di h c", di=4, h=2)
            ident16 = cmt[:, 4096:4224]

            # ACT table warm-up so the ~2.7us exp table load happens now
            wrm32 = cst.tile([128, 16], FP32, tag="wrm32")
            wrm16 = cst.tile([128, 16], FP16, tag="wrm16")
            nc.vector.memset(wrm32[:], 0.0)
            nc.scalar.activation(wrm16[:], wrm32[:], EXP, scale=SCALE)

            # ---------- staging tiles ----------
            qt = cst.tile([128, S], FP16, tag="qt")   # head h at partitions h*64..
            kt = cst.tile([128, S], FP16, tag="kt")
            vaug = cst.tile([128, NB, NHEAD, 66], FP16, tag="vaug")
            nc.vector.memset(vaug[:, :, :, 64:65], 1.0)

            nat16 = {
                "k": cst.tile([128, NB, DCORE], FP16, tag="nat16k",
                              name="nat16k"),
                "q": cst.tile([128, NB, DCORE], FP16, tag="nat16q",
                              name="nat16q"),
            }
            scr = {
                "k": dram.tile([S, DCORE], FP16, tag="scrk", name="scrk"),
                "q": dram.tile([S, DCORE], FP16, tag="scrq", name="scrq"),
            }
            srcs = {"k": k_d, "q": q_d}
            queues = {"k": nc.sync, "q": nc.gpsimd}
            src_r = {
                n: srcs[n].ap().rearrange("(n p) d -> p n d", p=128)
                for n in ("k", "q")
            }
            v_r = v_d.ap().rearrange("(n p) d -> p n d", p=128)

            def load_group(name, blk0, nblk):
                eng = queues[name]
                n32 = nat.tile([128, nblk, DCORE], FP32, tag="n32",
                               name=f"n32_{name}_{blk0}")
                sl = slice(blk0, blk0 + nblk)
                eng.dma_start(n32[:], src_r[name][:, sl, :])
                nc.vector.tensor_copy(nat16[name][:, sl, :], n32[:])

            def load_v_group(blk0, nblk):
                n32 = nat.tile([128, nblk, DCORE], FP32, tag="n32",
                               name=f"n32_v_{blk0}")
                sl = slice(blk0, blk0 + nblk)
                nc.gpsimd.dma_start(n32[:], v_r[:, sl, :])
                for h in range(NHEAD):
                    nc.vector.tensor_copy(
                        vaug[:, sl, h, 0:64], n32[:, :, h * 64:(h + 1) * 64]
                    )

            def boot(name, blk, h, copy_eng):
                """PE-transpose one [128, 64] block into kt/qt."""
                pt = ps_tp.tile([64, 130], FP16, tag="t",
                                name=f"bt_{name}_{blk}_{h}")
                nc.tensor.transpose(
                    pt[:, 0:128], nat16[name][:, blk, h * 64:(h + 1) * 64],
                    ident16,
                )
                dst = (kt if name == "k" else qt)
                if copy_eng == "s":
                    nc.scalar.copy(
                        dst[h * 64:(h + 1) * 64, blk * 128:(blk + 1) * 128],
                        pt[:, 0:128],
                    )
                else:
                    nc.vector.tensor_copy(
                        dst[h * 64:(h + 1) * 64, blk * 128:(blk + 1) * 128],
                        pt[:, 0:128],
                    )

            def xbar_group(name, blk0, nblk):
                eng = nc.sync    # xbar transpose only runs on HWDGE queues
                sl = slice(blk0, blk0 + nblk)
                rows = slice(blk0 * 128, (blk0 + nblk) * 128)
                scr_r = scr[name][:].rearrange("(n p) d -> p n d", p=128)
                eng.dma_start(scr_r[:, sl, :], nat16[name][:, sl, :])
                eng.dma_start_transpose(
                    out=(kt if name == "k" else qt)[:, rows],
                    in_=scr[name][rows, :],
                )

            # ---------- setup schedule (need order) ----------
            load_group("k", 0, 4)
            load_group("q", 0, 4)
            load_v_group(0, 4)
            for blk in range(4):
                for h in range(NHEAD):
                    boot("k", blk, h, "s")
                    boot("q", blk, h, "v")
            load_group("k", 4, 4)
            load_group("q", 4, 4)
            load_v_group(4, 4)
            for blk in range(4, 8):
                for h in range(NHEAD):
                    boot("k", blk, h, "s")
                    boot("q", blk, h, "v")
            for g in range(3):
                b0 = 8 + g * 8
                load_group("k", b0, 8)
                load_group("q", b0, 8)
                load_v_group(8 + g * 12, 12) if g < 2 else None
                xbar_group("k", b0, 8)
                xbar_group("q", b0, 8)

            # ---------- main loop ----------
            o_pools = (ps_o0, ps_o1)
            for j in range(NQC):
                nk = 4 * j + 4
                o_accs = [
                    o_pools[h].tile([65, QC], FP32, tag=f"oacc{h}",
                                    name=f"oacc{h}_{j}")
                    for h in range(NHEAD)
                ]
                for i in range(nk):
                    s_t = ps_s.tile([128, 2 * QC], FP32, tag="s",
                                    name=f"s_{j}_{i}")
                    for h in range(NHEAD):   # concurrent PE row groups
                        hp = slice(h * 64, (h + 1) * 64)
                        nc.tensor.matmul(
                            s_t[:, h * QC:(h + 1) * QC],
                            kt[hp, i * 128:(i + 1) * 128],
                            qt[hp, j * QC:(j + 1) * QC],
                            start=True, stop=True,
                        )
                    p_t = pp.tile([128, NHEAD, QC], FP16, tag="p")
                    di = i - 4 * j
                    # engine choice for the exp
                    if load["s"] + COST_S_EXP <= load["v"] + COST_V_EXP:
                        load["s"] += COST_S_EXP
                        nc.scalar.activation(
                            p_t[:], s_t[:], EXP, scale=SCALE
                        )
                    else:
                        load["v"] += COST_V_EXP
                        nc.vector.tensor_scalar(
                            p_t[:].bitcast(I16), s_t[:], SCH_C1, SCH_C2,
                            mybir.AluOpType.mult, mybir.AluOpType.add,
                        )
                    if di >= 0:   # diagonal block: zero the masked wedge
                        w = min(128 * (di + 1), QC)
                        load["v"] += MASK_COST[di]
                        nc.vector.tensor_tensor(
                            p_t[:, :, 0:w], p_t[:, :, 0:w],
                            mm[:, di, :, 0:w],
                            mybir.AluOpType.mult,
                        )
                    for h in range(NHEAD):
                        nc.tensor.matmul(
                            o_accs[h][:],
                            vaug[:, i, h, 0:65],
                            p_t[:, h, :],
                            start=(i == 0), stop=(i == nk - 1),
                        )
                # ---------- epilogue for this q-chunk ----------
                for h in range(NHEAD):
                    o_sb = ep.tile([65, QC], FP32, tag="osb")
                    if load["s"] + COST_COPY_S <= load["v"] + COST_COPY_V:
                        load["s"] += COST_COPY_S
                        nc.scalar.copy(o_sb[:], o_accs[h][:])
                    else:
                        load["v"] += COST_COPY_V
                        nc.vector.tensor_copy(o_sb[:], o_accs[h][:])
                    ps_t = ps_tp.tile([128, 4, 65], FP32, tag="t",
                                      name=f"pst_{j}_{h}")
                    for t in range(4):
                        nc.tensor.transpose(
                            ps_t[:, t, :], o_sb[:, t * 128:(t + 1) * 128],
                            ci[0:65, 0:65],
                        )
                    rec = ep.tile([128, 4], FP32, tag="rec")
                    nc.vector.reciprocal(rec[:], ps_t[:, :, 64])
                    ob = ep.tile([128, 4, 64], FP32, tag="ob")
                    nc.vector.tensor_tensor(
                        ob[:], ps_t[:, :, 0:64],
                        rec[:].rearrange("p (t o) -> p t o", o=1).broadcast_to(
                            (128, 4, 64)),
                        mybir.AluOpType.mult,
                    )
                    load["v"] += 560.0
                    qrow = j * QC
                    nc.sync.dma_start(
                        o_d.ap()[qrow:qrow + QC, h * 64:(h + 1) * 64]
                        .rearrange("(t p) d -> p t d", p=128),
                        ob[:],
                    )

    nc.compile()
    return nc


def kernel(**inputs) -> np.ndarray:
    from concourse.bass_utils import run_bass_kernel_spmd

    global _CACHED_NC, LAST_RES
    query = np.asarray(inputs["query"], dtype=np.float32)
    key = np.asarray(inputs["key"], dtype=np.float32)
    value = np.asarray(inputs["value"], dtype=np.float32)
    assert int(inputs["num_head"]) == 16 and int(inputs["dim_head"]) == 64
    b, s, d = query.shape
    assert (b, s, d) == (1, S, 1024)

    if _CACHED_NC is None:
        _CACHED_NC = build_attn()
    nc = _CACHED_NC

    cm, ci = _build_consts()
    in_maps = []
    for c in range(8):
        cols = slice(c * DCORE, (c + 1) * DCORE)
        in_maps.append({
            "q": np.ascontiguousarray(query[0][:, cols]),
            "k": np.ascontiguousarray(key[0][:, cols]),
            "v": np.ascontiguousarray(value[0][:, cols]),
            "cm": cm,
            "ci": ci,
        })
    res = run_bass_kernel_spmd(nc, in_maps, list(range(8)), trace=TRACE)
    LAST_RES = res
    out = np.concatenate([res.results[c]["o"] for c in range(8)], axis=1)
    return out[None].astype(np.float32)


# revision 8
# speedup vs baseline: 1.0127x; 1.0127x over previous
"""Causal multi-head attention (B=1, S=4096, H=16, Dh=64) on 8 TRN2
NeuronCores, head-parallel (2 heads per core), flash-style (scores never
touch HBM).

Per-core SPMD program (q/k/v [4096, 128] fp32 = 2 heads side by side,
output o [4096, 128] fp32):
  - Scores transposed, S^T[k, q] = K @ Q^T, contraction dh=64, fp16; the
    two heads sit at partitions 0..63 / 64..127 so their score matmuls
    land on different PE row groups and run concurrently.
  - exp() split across TWO engines, balanced at build time:
      * ScalarE ACT: p = exp(s/8) -> fp16           (~1.11us / block)
      * VectorE DVE: Schraudolph bit-trick exp      (~1.22us / block)
        i16 = round(s * (2^10*log2e/8) + (15*2^10 - 44)); bitcast fp16.
        ~3% sawtooth rel err; the softmax ratio cancels most of it
        (measured end-to-end ~4e-3 vs the 2e-2 budget).
  - PE software pipelining: scores run LOOK=2 blocks ahead of the
    exp->AV consumers in the PE FIFO so the PE never waits on exp;
    scores PSUM pool is 3 deep (6 banks) + 2 banks for the two o_acc.
  - Causality at block granularity: upper-triangle k-blocks skipped;
    diagonal blocks multiplied by 0/1 fp16 masks (one [128, 2, w]
    tensor_tensor covers both heads, w bounded per sub-diagonal).
  - AV: out^T[dh, q] per head accumulated in PSUM via lhsT = V_aug
    [128, 65] = [V | ones]/16 -> row 64 = softmax denominator/16
    (1/16 keeps everything comfortably inside fp16 for the epilogue).
  - Epilogue per (chunk, head): copy o_acc to fp16 SBUF, xbar DMA
    transpose ([80, 512] -> [128, 4, 80], row q lands at partition
    q%128, slot q//128), reciprocal of the denominator column, one
    broadcast multiply, DMA out.  No PE or PSUM involvement.
  - Masks + identity are DMA'd in as precomputed constants.
  - Setup: fp32 loads -> fp16 cast -> DRAM round trip -> xbar DMA
    transpose, in need-ordered groups (first 2 chunks use 4-block
    groups); k on sync queue, q/v loads on gpsimd, late casts on
    gpsimd to keep VectorE free for exp.
"""
import numpy as np

import concourse.bass as bass
import concourse.tile as tile
import concourse.mybir as mybir
from concourse import bacc

FP32 = mybir.dt.float32
FP16 = mybir.dt.float16
I16 = mybir.dt.int16

S = 4096
DH = 64
NHEAD = 2          # heads per core
DCORE = NHEAD * DH
NB = S // 128      # 32 k-blocks
QC = 512
NQC = S // QC      # 8 q-chunks
SCALE = 1.0 / 8.0
VSCALE = 1.0 / 16.0
EXP = mybir.ActivationFunctionType.Exp

# Schraudolph constants (fp16 target): i16 = s * C1 + C2, bitcast fp16.
SCH_C1 = float(1024.0 * 1.4426950408889634 * SCALE)
SCH_C2 = float(15 * 1024 - 44.0)

LOOK = 2           # scores lookahead (blocks) in the PE stream

_CACHED_NC = None
TRACE = False
LAST_RES = None


def _build_consts():
    """Host-side constant tensor DMA'd into SBUF at kernel start."""
    # wide diagonal masks: cm[:, di*1024 + h*512 + c]; keep iff p <= c - 128*di
    p = np.arange(128)[:, None]
    c = np.arange(512)[None, :]
    masks = np.zeros((128, 4, 2, 512), dtype=np.float16)
    for di in range(4):
        m = (p <= c - 128 * di).astype(np.float16)
        masks[:, di, 0, :] = m
        masks[:, di, 1, :] = m
    return masks.reshape(128, 4096)


def build_attn():
    nc = bacc.Bacc(None, target_bir_lowering=False, debug=False)
    q_d = nc.dram_tensor("q", [S, DCORE], FP32, kind="ExternalInput")
    k_d = nc.dram_tensor("k", [S, DCORE], FP32, kind="ExternalInput")
    v_d = nc.dram_tensor("v", [S, DCORE], FP32, kind="ExternalInput")
    cm_d = nc.dram_tensor("cm", [128, 4096], FP16, kind="ExternalInput")
    o_d = nc.dram_tensor("o", [S, DCORE], FP32, kind="ExternalOutput")

    # build-time engine load balancer (ns estimates from HW microbench)
    load = {"s": 500.0, "v": 16000.0}
    COST_S_EXP, COST_V_EXP = 1150.0, 1250.0
    MASK_COST = (260.0, 400.0, 530.0, 660.0)
    COST_COPY_S, COST_COPY_V = 620.0, 700.0

    with tile.TileContext(nc) as tc:
        with (
            tc.tile_pool(name="cst", bufs=1) as cst,
            tc.tile_pool(name="nats", bufs=3) as nats,
            tc.tile_pool(name="natb", bufs=3) as natb,
            tc.tile_pool(name="pp", bufs=6) as pp,
            tc.tile_pool(name="ep", bufs=4) as ep,
            tc.tile_pool(name="dram", bufs=1, space="DRAM") as dram,
            tc.tile_pool(name="ps_s", bufs=3, space="PSUM") as ps_s,
            tc.tile_pool(name="ps_o0", bufs=1, space="PSUM") as ps_o0,
            tc.tile_pool(name="ps_o1", bufs=1, space="PSUM") as ps_o1,
        ):
            # ---------- constants (DMA'd, not computed) ----------
            cmt = cst.tile([128, 4096], FP16, tag="cmt")
            nc.sync.dma_start(cmt[:], cm_d.ap())
            mm = cmt[:].rearrange("p (di h c) -> p di h c", di=4, h=2)

            # ACT table warm-up so the ~2.7us exp table load happens now
            wrm32 = cst.tile([128, 16], FP32, tag="wrm32")
            wrm16 = cst.tile([128, 16], FP16, tag="wrm16")
            nc.vector.memset(wrm32[:], 0.0)
            nc.scalar.activation(wrm16[:], wrm32[:], EXP, scale=SCALE)

            # ---------- staging ----------
            qt = cst.tile([128, S], FP16, tag="qt")   # head h at partitions h*64..
            kt = cst.tile([128, S], FP16, tag="kt")
            vaug = cst.tile([128, NB, NHEAD, 66], FP16, tag="vaug")
            nc.vector.memset(vaug[:, :, :, 64:65], VSCALE)

            nat16 = {
                "k": cst.tile([128, NB, DCORE], FP16, tag="nat16k",
                              name="nat16k"),
                "q": cst.tile([128, NB, DCORE], FP16, tag="nat16q",
                              name="nat16q"),
            }
            scr = {
                "k": dram.tile([S, DCORE], FP16, tag="scrk", name="scrk"),
                "q": dram.tile([S, DCORE], FP16, tag="scrq", name="scrq"),
            }
            srcs = {"k": k_d, "q": q_d}
            load_q = {"k": nc.sync, "q": nc.gpsimd}
            src_r = {
                n: srcs[n].ap().rearrange("(n p) d -> p n d", p=128)
                for n in ("k", "q")
            }
            v_r = v_d.ap().rearrange("(n p) d -> p n d", p=128)

            def kq_group(name, blk0, nblk, cast_eng):
                pool = nats if nblk == 4 else natb
                n32 = pool.tile([128, nblk, DCORE], FP32, tag="n32",
                                name=f"n32_{name}_{blk0}")
                sl = slice(blk0, blk0 + nblk)
                rows = slice(blk0 * 128, (blk0 + nblk) * 128)
                load_q[name].dma_start(n32[:], src_r[name][:, sl, :])
                cast_eng.tensor_copy(nat16[name][:, sl, :], n32[:])
                scr_r = scr[name][:].rearrange("(n p) d -> p n d", p=128)
                nc.sync.dma_start(scr_r[:, sl, :], nat16[name][:, sl, :])
                nc.sync.dma_start_transpose(
                    out=(kt if name == "k" else qt)[:, rows],
                    in_=scr[name][rows, :],
                )

            def v_group(blk0, nblk, cast_eng):
                pool = nats if nblk == 4 else natb
                n32 = pool.tile([128, nblk, DCORE], FP32, tag="n32",
                                name=f"n32_v_{blk0}")
                sl = slice(blk0, blk0 + nblk)
                nc.gpsimd.dma_start(n32[:], v_r[:, sl, :])
                for h in range(NHEAD):
                    cast_eng.tensor_scalar_mul(
                        vaug[:, sl, h, 0:64],
                        n32[:, :, h * 64:(h + 1) * 64], VSCALE,
                    )

            # ---------- setup schedule (need order) ----------
            kq_group("k", 0, 4, nc.vector)
            kq_group("q", 0, 4, nc.vector)
            v_group(0, 4, nc.vector)
            kq_group("k", 4, 4, nc.vector)
            kq_group("q", 4, 4, nc.vector)
            v_group(4, 4, nc.vector)
            for g in range(3):
                b0 = 8 + g * 8
                kq_group("k", b0, 8, nc.gpsimd)
                kq_group("q", b0, 8, nc.gpsimd)
                v_group(b0, 8, nc.gpsimd)

            # ---------- main loop (flat, software-pipelined) ----------
            o_pools = (ps_o0, ps_o1)
            blist = [(j, i) for j in range(NQC) for i in range(4 * j + 4)]
            p_tiles = {}

            def emit_scores(j, i):
                s_t = ps_s.tile([128, 2 * QC], FP32, tag="s",
                                name=f"s_{j}_{i}")
                for h in range(NHEAD):   # concurrent PE row groups
                    hp = slice(h * 64, (h + 1) * 64)
                    nc.tensor.matmul(
                        s_t[:, h * QC:(h + 1) * QC],
                        kt[hp, i * 128:(i + 1) * 128],
                        qt[hp, j * QC:(j + 1) * QC],
                        start=True, stop=True,
                    )
                return s_t

            def emit_body(j, i, s_t, o_accs):
                nk = 4 * j + 4
                p_t = pp.tile([128, NHEAD, QC], FP16, tag="p",
                              name=f"p_{j}_{i}")
                if load["s"] + COST_S_EXP <= load["v"] + COST_V_EXP:
                    load["s"] += COST_S_EXP
                    nc.scalar.activation(p_t[:], s_t[:], EXP, scale=SCALE)
                else:
                    load["v"] += COST_V_EXP
                    nc.vector.tensor_scalar(
                        p_t[:].bitcast(I16), s_t[:], SCH_C1, SCH_C2,
                        mybir.AluOpType.mult, mybir.AluOpType.add,
                    )
                di = i - 4 * j
                if di >= 0:   # diagonal block: zero the masked wedge
                    w = min(128 * (di + 1), QC)
                    load["v"] += MASK_COST[di]
                    nc.vector.tensor_tensor(
                        p_t[:, :, 0:w], p_t[:, :, 0:w],
                        mm[:, di, :, 0:w],
                        mybir.AluOpType.mult,
                    )
                for h in range(NHEAD):
                    nc.tensor.matmul(
                        o_accs[h][:],
                        vaug[:, i, h, 0:65],
                        p_t[:, h, :],
                        start=(i == 0), stop=(i == nk - 1),
                    )

            def emit_epilogue(j, o_accs):
                for h in range(NHEAD):
                    o_sb = ep.tile([80, QC], FP16, tag="osb",
                                   name=f"osb_{j}_{h}")
                    if load["s"] + COST_COPY_S <= load["v"] + COST_COPY_V:
                        load["s"] += COST_COPY_S
                        nc.scalar.copy(o_sb[0:65, :], o_accs[h][:])
                    else:
                        load["v"] += COST_COPY_V
                        nc.vector.tensor_copy(o_sb[0:65, :], o_accs[h][:])
                    ot = ep.tile([128, 4, 80], FP16, tag="ot",
                                 name=f"ot_{j}_{h}")
                    nc.sync.dma_start_transpose(out=ot[:], in_=o_sb[:])
                    rec = ep.tile([128, 4], FP32, tag="rec",
                                  name=f"rec_{j}_{h}")
                    nc.vector.reciprocal(rec[:], ot[:, :, 64])
                    ob = ep.tile([128, 4, 64], FP32, tag="ob",
                                 name=f"ob_{j}_{h}")
                    nc.vector.tensor_tensor(
                        ob[:], ot[:, :, 0:64],
                        rec[:].rearrange("p (t o) -> p t o", o=1)
                        .broadcast_to((128, 4, 64)),
                        mybir.AluOpType.mult,
                    )
                    load["v"] += 420.0
                    qrow = j * QC
                    nc.sync.dma_start(
                        o_d.ap()[qrow:qrow + QC, h * 64:(h + 1) * 64]
                        .rearrange("(t p) d -> p t d", p=128),
                        ob[:],
                    )

            o_accs_of = {}
            s_of = {}

            def body_and_maybe_epilogue(j, i):
                emit_body(j, i, s_of.pop((j, i)), o_accs_of[j])
                if i == 4 * j + 3:
                    emit_epilogue(j, o_accs_of.pop(j))

            for n, (j, i) in enumerate(blist):
                if i == 0:
                    o_accs_of[j] = [
                        o_pools[h].tile([65, QC], FP32, tag=f"oacc{h}",
                                        name=f"oacc{h}_{j}")
                        for h in range(NHEAD)
                    ]
                s_of[(j, i)] = emit_scores(j, i)
                if n >= LOOK:
                    body_and_maybe_epilogue(*blist[n - LOOK])
            for n in range(len(blist) - LOOK, len(blist)):
                body_and_maybe_epilogue(*blist[n])

    nc.compile()
    return nc


def kernel(**inputs) -> np.ndarray:
    from concourse.bass_utils import run_bass_kernel_spmd

    global _CACHED_NC, LAST_RES
    query = np.asarray(inputs["query"], dtype=np.float32)
    key = np.asarray(inputs["key"], dtype=np.float32)
    value = np.asarray(inputs["value"], dtype=np.float32)
    assert int(inputs["num_head"]) == 16 and int(inputs["dim_head"]) == 64
    b, s, d = query.shape
    assert (b, s, d) == (1, S, 1024)

    if _CACHED_NC is None:
        _CACHED_NC = build_attn()
    nc = _CACHED_NC

    cm = _build_consts()
    in_maps = []
    for c in range(8):
        cols = slice(c * DCORE, (c + 1) * DCORE)
        in_maps.append({
            "q": np.ascontiguousarray(query[0][:, cols]),
            "k": np.ascontiguousarray(key[0][:, cols]),
            "v": np.ascontiguousarray(value[0][:, cols]),
            "cm": cm,
        })
    res = run_bass_kernel_spmd(nc, in_maps, list(range(8)), trace=TRACE)
    LAST_RES = res
    out = np.concatenate([res.results[c]["o"] for c in range(8)], axis=1)
    return out[None].astype(np.float32)


# revision 13
# speedup vs baseline: 1.0753x; 1.0619x over previous
"""Causal multi-head attention (B=1, S=4096, H=16, Dh=64) on 8 TRN2
NeuronCores, head-parallel (2 heads per core), flash-style (scores never
touch HBM).

Per-core SPMD program (q/k/v [4096, 128] fp32 = 2 heads side by side,
output o [4096, 128] fp32):
  - Scores transposed, S^T[k, q] = K @ Q^T, contraction dh=64, fp16; the
    two heads sit at partitions 0..63 / 64..127 so their score matmuls
    land on different PE row groups and run concurrently.
  - exp() split across TWO engines, balanced at build time:
      * ScalarE ACT: p = exp(s/8) -> fp16           (~1.11us / block)
      * VectorE DVE: Schraudolph bit-trick exp      (~1.22us / block)
        i16 = round(s * (2^10*log2e/8) + (15*2^10 - 44)); bitcast fp16.
        ~3% sawtooth rel err; the softmax ratio cancels most of it
        (measured end-to-end ~4e-3 vs the 2e-2 budget).
  - PE software pipelining: scores run LOOK=2 blocks ahead of the
    exp->AV consumers in the PE FIFO so the PE never waits on exp;
    scores PSUM pool is 3 deep (6 banks) + 2 banks for the two o_acc.
  - Causality at block granularity: upper-triangle k-blocks skipped;
    diagonal blocks multiplied by 0/1 fp16 masks (one [128, 2, w]
    tensor_tensor covers both heads, w bounded per sub-diagonal).
  - AV: out^T[dh, q] per head accumulated in PSUM via lhsT = V_aug
    [128, 65] = [V | ones]/16 -> row 64 = softmax denominator/16
    (1/16 keeps everything comfortably inside fp16 for the epilogue).
  - Epilogue per (chunk, head): copy o_acc to fp16 SBUF, xbar DMA
    transpose ([80, 512] -> [128, 4, 80], row q lands at partition
    q%128, slot q//128), reciprocal of the denominator column, one
    broadcast multiply, DMA out.  No PE or PSUM involvement.
  - Masks + identity are DMA'd in as precomputed constants.
  - Setup: fp32 loads -> fp16 cast -> DRAM round trip -> xbar DMA
    transpose, in need-ordered groups (first 2 chunks use 4-block
    groups); k on sync queue, q/v loads on gpsimd, late casts on
    gpsimd to keep VectorE free for exp.
"""
import numpy as np

import concourse.bass as bass
import concourse.tile as tile
import concourse.mybir as mybir
from concourse import bacc

FP32 = mybir.dt.float32
FP16 = mybir.dt.float16
I16 = mybir.dt.int16

S = 4096
DH = 64
NHEAD = 2          # heads per core
DCORE = NHEAD * DH
NB = S // 128      # 32 k-blocks
QC = 512
NQC = S // QC      # 8 q-chunks
SCALE = 1.0 / 8.0
VSCALE = 1.0 / 16.0
EXP = mybir.ActivationFunctionType.Exp

# Schraudolph constants (fp16 target): i16 = s * C1 + C2, bitcast fp16.
SCH_C1 = float(1024.0 * 1.4426950408889634 * SCALE)
SCH_C2 = float(15 * 1024 - 44.0)

LOOK = 2           # scores lookahead (blocks) in the PE stream

_CACHED_NC = None
TRACE = False
LAST_RES = None


def _build_consts():
    """Host-side constant tensor DMA'd into SBUF at kernel start."""
    # wide diagonal masks: cm[:, di*1024 + h*512 + c]; keep iff p <= c - 128*di
    p = np.arange(128)[:, None]
    c = np.arange(512)[None, :]
    masks = np.zeros((128, 4, 2, 512), dtype=np.float16)
    for di in range(4):
        m = (p <= c - 128 * di).astype(np.float16)
        masks[:, di, 0, :] = m
        masks[:, di, 1, :] = m
    ones = np.full((128, 64), VSCALE, dtype=np.float16)
    return np.concatenate([masks.reshape(128, 4096), ones], axis=1)


def build_attn():
    nc = bacc.Bacc(None, target_bir_lowering=False, debug=False)
    q_d = nc.dram_tensor("q", [S, DCORE], FP32, kind="ExternalInput")
    k_d = nc.dram_tensor("k", [S, DCORE], FP32, kind="ExternalInput")
    v_d = nc.dram_tensor("v", [S, DCORE], FP32, kind="ExternalInput")
    cm_d = nc.dram_tensor("cm", [128, 4096 + 64], FP16, kind="ExternalInput")
    o_d = nc.dram_tensor("o", [S, DCORE], FP32, kind="ExternalOutput")

    # build-time engine load balancer (ns estimates from HW microbench)
    load = {"s": 500.0, "v": 500.0}
    COST_S_EXP, COST_V_EXP = 1150.0, 1250.0
    MASK_COST = (260.0, 400.0, 530.0, 660.0)
    COST_COPY_S, COST_COPY_V = 620.0, 700.0

    def balanced(cost_s, cost_v):
        if load["s"] + cost_s <= load["v"] + cost_v:
            load["s"] += cost_s
            return "s"
        load["v"] += cost_v
        return "v"

    with tile.TileContext(nc) as tc:
        with (
            tc.tile_pool(name="cst", bufs=1) as cst,
            tc.tile_pool(name="nats", bufs=3) as nats,
            tc.tile_pool(name="natb", bufs=3) as natb,
            tc.tile_pool(name="pp", bufs=6) as pp,
            tc.tile_pool(name="ep", bufs=4) as ep,
            tc.tile_pool(name="dram", bufs=1, space="DRAM") as dram,
            tc.tile_pool(name="ps_s", bufs=3, space="PSUM") as ps_s,
            tc.tile_pool(name="ps_o0", bufs=1, space="PSUM") as ps_o0,
            tc.tile_pool(name="ps_o1", bufs=1, space="PSUM") as ps_o1,
        ):
            # ---------- constants (DMA'd on scalar queue, not computed) ----------
            cmt = cst.tile([128, 4096 + 64], FP16, tag="cmt")
            nc.scalar.dma_start(cmt[:], cm_d.ap())
            mm = cmt[:, 0:4096].rearrange("p (di h c) -> p di h c", di=4, h=2)

            # ACT table warm-up so the ~2.7us exp table load happens now
            wrm16 = cst.tile([128, 16], FP16, tag="wrm16")
            nc.scalar.activation(wrm16[:], cmt[:, 0:16], EXP, scale=SCALE)

            # ---------- staging ----------
            qt = cst.tile([128, S], FP16, tag="qt")   # head h at partitions h*64..
            kt = cst.tile([128, S], FP16, tag="kt")
            vaug = cst.tile([128, NB, NHEAD, 66], FP16, tag="vaug")
            # ones/16 column via strided DMA from the constant block
            nc.scalar.dma_start(
                vaug[:, :, :, 64:65],
                cmt[:, 4096:4160].rearrange("p (a b o) -> p a b o", a=NB, b=2),
            )

            nat16 = {
                "k": cst.tile([128, NB, DCORE], FP16, tag="nat16k",
                              name="nat16k"),
                "q": cst.tile([128, NB, DCORE], FP16, tag="nat16q",
                              name="nat16q"),
            }
            scr = {
                "k": dram.tile([S, DCORE], FP16, tag="scrk", name="scrk"),
                "q": dram.tile([S, DCORE], FP16, tag="scrq", name="scrq"),
            }
            srcs = {"k": k_d, "q": q_d}
            load_q = {"k": nc.sync, "q": nc.gpsimd}
            src_r = {
                n: srcs[n].ap().rearrange("(n p) d -> p n d", p=128)
                for n in ("k", "q")
            }
            v_r = v_d.ap().rearrange("(n p) d -> p n d", p=128)

            def kq_group(name, blk0, nblk):
                pool = nats if nblk == 4 else natb
                n32 = pool.tile([128, nblk, DCORE], FP32, tag="n32",
                                name=f"n32_{name}_{blk0}")
                sl = slice(blk0, blk0 + nblk)
                rows = slice(blk0 * 128, (blk0 + nblk) * 128)
                load_q[name].dma_start(n32[:], src_r[name][:, sl, :])
                ns = nblk * DCORE
                if balanced(172 + ns / 1.2, 58 + ns / 1.92) == "s":
                    nc.scalar.copy(nat16[name][:, sl, :], n32[:])
                else:
                    nc.vector.tensor_copy(nat16[name][:, sl, :], n32[:])
                scr_r = scr[name][:].rearrange("(n p) d -> p n d", p=128)
                nc.sync.dma_start(scr_r[:, sl, :], nat16[name][:, sl, :])
                nc.sync.dma_start_transpose(
                    out=(kt if name == "k" else qt)[:, rows],
                    in_=scr[name][rows, :],
                )

            def v_group(blk0, nblk):
                pool = nats if nblk == 4 else natb
                n32 = pool.tile([128, nblk, DCORE], FP32, tag="n32",
                                name=f"n32_v_{blk0}")
                sl = slice(blk0, blk0 + nblk)
                nc.gpsimd.dma_start(n32[:], v_r[:, sl, :])
                ns = nblk * DH
                for h in range(NHEAD):
                    src = n32[:, :, h * 64:(h + 1) * 64]
                    dst = vaug[:, sl, h, 0:64]
                    if balanced(172 + ns / 1.2, 58 + ns / 1.92) == "s":
                        nc.scalar.mul(dst, src, VSCALE)
                    else:
                        nc.vector.tensor_scalar_mul(dst, src, VSCALE)

            # ---------- setup schedule (need order) ----------
            kq_group("k", 0, 4)
            kq_group("q", 0, 4)
            v_group(0, 4)
            kq_group("k", 4, 4)
            kq_group("q", 4, 4)
            v_group(4, 4)
            for g in range(3):
                b0 = 8 + g * 8
                kq_group("k", b0, 8)
                kq_group("q", b0, 8)
                v_group(b0, 8)

            # ---------- main loop (flat, software-pipelined) ----------
            o_pools = (ps_o0, ps_o1)
            blist = [(j, i) for j in range(NQC) for i in range(4 * j + 4)]
            p_tiles = {}

            def emit_scores(j, i):
                s_t = ps_s.tile([128, 2 * QC], FP32, tag="s",
                                name=f"s_{j}_{i}")
                for h in range(NHEAD):   # concurrent PE row groups
                    hp = slice(h * 64, (h + 1) * 64)
                    nc.tensor.matmul(
                        s_t[:, h * QC:(h + 1) * QC],
                        kt[hp, i * 128:(i + 1) * 128],
                        qt[hp, j * QC:(j + 1) * QC],
                        start=True, stop=True,
                    )
                return s_t

            def emit_body(j, i, s_t, o_accs):
                nk = 4 * j + 4
                p_t = pp.tile([128, NHEAD, QC], FP16, tag="p",
                              name=f"p_{j}_{i}")
                if load["s"] + COST_S_EXP <= load["v"] + COST_V_EXP:
                    load["s"] += COST_S_EXP
                    nc.scalar.activation(p_t[:], s_t[:], EXP, scale=SCALE)
                else:
                    load["v"] += COST_V_EXP
                    nc.vector.tensor_scalar(
                        p_t[:].bitcast(I16), s_t[:], SCH_C1, SCH_C2,
                        mybir.AluOpType.mult, mybir.AluOpType.add,
                    )
                di = i - 4 * j
                if di >= 0:   # diagonal block: zero the masked wedge
                    w = min(128 * (di + 1), QC)
                    load["v"] += MASK_COST[di]
                    nc.vector.tensor_tensor(
                        p_t[:, :, 0:w], p_t[:, :, 0:w],
                        mm[:, di, :, 0:w],
                        mybir.AluOpType.mult,
                    )
                for h in range(NHEAD):
                    nc.tensor.matmul(
                        o_accs[h][:],
                        vaug[:, i, h, 0:65],
                        p_t[:, h, :],
                        start=(i == 0), stop=(i == nk - 1),
                    )

            def emit_epilogue(j, o_accs):
                for h in range(NHEAD):
                    o_sb = ep.tile([80, QC], FP16, tag="osb",
                                   name=f"osb_{j}_{h}")
                    if load["s"] + COST_COPY_S <= load["v"] + COST_COPY_V:
                        load["s"] += COST_COPY_S
                        nc.scalar.copy(o_sb[0:65, :], o_accs[h][:])
                    else:
                        load["v"] += COST_COPY_V
                        nc.vector.tensor_copy(o_sb[0:65, :], o_accs[h][:])
                    ot = ep.tile([128, 4, 80], FP16, tag="ot",
                                 name=f"ot_{j}_{h}")
                    nc.sync.dma_start_transpose(out=ot[:], in_=o_sb[:])
                    rec = ep.tile([128, 4], FP32, tag="rec",
                                  name=f"rec_{j}_{h}")
                    nc.vector.reciprocal(rec[:], ot[:, :, 64])
                    ob = ep.tile([128, 4, 64], FP32, tag="ob",
                                 name=f"ob_{j}_{h}")
                    load["v"] += 180.0
                    for t in range(4):
                        nc.vector.tensor_scalar_mul(
                            ob[:, t, :], ot[:, t, 0:64], rec[:, t:t + 1]
                        )
                    load["v"] += 4 * 160.0
                    qrow = j * QC
                    nc.sync.dma_start(
                        o_d.ap()[qrow:qrow + QC, h * 64:(h + 1) * 64]
                        .rearrange("(t p) d -> p t d", p=128),
                        ob[:],
                    )

            o_accs_of = {}
            s_of = {}

            def body_and_maybe_epilogue(j, i):
                emit_body(j, i, s_of.pop((j, i)), o_accs_of[j])
                if i == 4 * j + 3:
                    emit_epilogue(j, o_accs_of.pop(j))

            for n, (j, i) in enumerate(blist):
                if i == 0:
                    o_accs_of[j] = [
                        o_pools[h].tile([65, QC], FP32, tag=f"oacc{h}",
                                        name=f"oacc{h}_{j}")
                        for h in range(NHEAD)
                    ]
                s_of[(j, i)] = emit_scores(j, i)
                if n >= LOOK:
                    body_and_maybe_epilogue(*blist[n - LOOK])
            for n in range(len(blist) - LOOK, len(blist)):
                body_and_maybe_epilogue(*blist[n])

    nc.compile()
    return nc


def kernel(**inputs) -> np.ndarray:
    from concourse.bass_utils import run_bass_kernel_spmd

    global _CACHED_NC, LAST_RES
    query = np.asarray(inputs["query"], dtype=np.float32)
    key = np.asarray(inputs["key"], dtype=np.float32)
    value = np.asarray(inputs["value"], dtype=np.float32)
    assert int(inputs["num_head"]) == 16 and int(inputs["dim_head"]) == 64
    b, s, d = query.shape
    assert (b, s, d) == (1, S, 1024)

    if _CACHED_NC is None:
        _CACHED_NC = build_attn()
    nc = _CACHED_NC

    cm = _build_consts()
    in_maps = []
    for c in range(8):
        cols = slice(c * DCORE, (c + 1) * DCORE)
        in_maps.append({
            "q": np.ascontiguousarray(query[0][:, cols]),
            "k": np.ascontiguousarray(key[0][:, cols]),
            "v": np.ascontiguousarray(value[0][:, cols]),
            "cm": cm,
        })
    res = run_bass_kernel_spmd(nc, in_maps, list(range(8)), trace=TRACE)
    LAST_RES = res
    out = np.concatenate([res.results[c]["o"] for c in range(8)], axis=1)
    return out[None].astype(np.float32)
